# revision 52
# baseline (speedup 1.0000x reference)
"""JointCCSA loss kernel for 8 Trainium2 NeuronCores — circulant-triangle v3.

reference:
    dists = cdist(X, X)                                  (bs, bs)
    sa_loss = 0.5 * sum[ same_y & ds_lt ] dists / n_sa
    s_loss  = 0.5 * sum[ y_lt  & ds_lt ] relu(1 - dists) / n_s

Strategy (each unordered pair computed ONCE):
  * 16 row strips of 256.  Strip s pairs with col blocks (s+t) mod 16 for
    t=0..8 (t=8 only when s<8): every unordered block pair covered exactly
    once.  Core k owns strips k (9 blocks) and 15-k (8 blocks) -> 17 blocks
    = 4352 cols/core, uniform.  Host rotates columns per core so the device
    program is identical (SPMD).
  * Off-diagonal blocks use symmetrized masks f'(ci,cj)=f(ci,cj)+f(cj,ci)
    (counts both orderings); diagonal blocks the original ordered masks.
    All masks are rank-12 in the (y,ds) combo -> 12-wide matmuls.
  * fp8(e4m3) Gram: d2 = q8(-2x_i)@q8(x_j) + (sqhi_j+sqlo_j via a K=2
    ones-matmul folded into the same PSUM accumulation) + sq_i (ACT bias),
    where sq_i := -0.5*sum q8(-2x_i)*q8(x_i) exactly (f64 on host) so the
    diagonal lands on ~0 (+C0 guard) and sqrt never sees a negative.
  * dist = Sqrt(...) on ScalarE -> bf16; dmin = min(dist,1) on VectorE.
  * Mask reduce: Tsa/Ts accumulate over the two 128-row chunks into one
    PSUM bank via column-tiled concurrent matmuls (sa at col-group 0,
    s at col-group 32).  One scalar_tensor_tensor per span multiplies by
    the one-hot combo mask EE and row-reduces into acc[:, t].
  * Chunk-interleaved emission: mask-matmuls lag the Gram by one chunk so
    the PE never waits on the sqrt/min chain; host sums acc across cores.
"""

import numpy as np
import ml_dtypes
from contextlib import ExitStack

import concourse.bass as bass
import concourse.tile as tile
from concourse import mybir
from concourse.vector_clock import ScopedClock
from concourse.bass_utils import run_bass_kernel_spmd
from concourse.alu_op_type import AluOpType

BS = 4096
D = 512
NCORES = 8
NSTRIP = 16
SW = BS // NSTRIP            # 256 rows per strip
KCH = D // 128               # 4 contraction chunks
NCOLS = 17 * SW              # 4352 cols per core
C0 = 0.0625                  # sqrt-safety bias added into sq_i
F8 = ml_dtypes.float8_e4m3fn
BF16 = ml_dtypes.bfloat16
N_WARM = 6                   # HAM warmup matmuls during the DMA wait

# Per-strip span tables: (col offset rel. to strip, width, is_diag)
SPANS_A = [(0, 512, 0), (512, 512, 0), (1024, 512, 0), (1536, 512, 0),
           (2048, 256, 1)]
SPANS_B = [(0, 512, 0), (512, 512, 0), (1024, 512, 0), (1536, 256, 0),
           (1792, 256, 1)]
# chunks: groups of spans sharing one 2-bank d2 tile (ACT/min granularity)
CHUNKS_A = [(0, 1024, [0, 1]), (1024, 1024, [2, 3]), (2048, 256, [4])]
CHUNKS_B = [(0, 1024, [0, 1]), (1024, 768, [2, 3]), (1792, 256, [4])]
STRIPS = [(0, 2304, SPANS_A, CHUNKS_A), (2304, 2048, SPANS_B, CHUNKS_B)]
NT = 10                      # acc columns (one per span)


# ---------------------------------------------------------------------------
# Patch: this walrus build allows only ONE sync-wait on a CTRL-type (Drain)
# instruction; Tile's final drain aggregates many.  Spread them over
# single-wait SP nops.
def _patched_drain_and_barrier(self, tick_clock, wait_clock):
    nc = self.nc
    coll = nc.sync.nop(nofuse=True, hint="drain_wait_collector")
    wait_clock.add_sem_waits(coll.ins, ScopedClock({None: tick_clock.global_clock}))
    si = coll.ins.sync_info
    waits = list(si.on_wait) if si is not None else []
    if len(waits) > 1:
        si.on_wait = [waits[0]]
        for w in waits[1:]:
            n = nc.sync.nop(nofuse=True, hint="drain_wait_extra")
            n.ins.sync_info = mybir.SyncInfo(on_wait=[w], on_update=[])
    nc.sync.drain()
    nc.all_engine_barrier()
    assert self.sems is not None
    popped = nc._tile_sem_poison_stack.pop()
    assert popped is self._sem_poison
    nc.clear_and_free_semaphores(list(self.sems.allocated().values()))
    nc.all_engine_barrier()


tile.TileContext._drain_and_barrier = _patched_drain_and_barrier


def _split_waits(nc, maxw=1):
    """Hoist extra sync-waits from every instruction onto same-engine NoOps
    (this walrus build rejects instructions with more than ~1 wait)."""
    for fn in nc.m.functions:
        for blk in fn.blocks:
            newlist = []
            for inst in blk.instructions:
                si = getattr(inst, "sync_info", None)
                if si is not None and len(si.on_wait) > maxw:
                    waits = list(si.on_wait)
                    for i, w in enumerate(waits[maxw:]):
                        nop = mybir.InstNoOp(
                            name=f"{inst.name}-wsplit{i}",
                            sync_info=mybir.SyncInfo(on_wait=[w], on_update=[]),
                            bass_nofuse=True,
                            engine=inst.engine,
                        )
                        nc.register_instruction(nop)
                        newlist.append(nop)
                    si.on_wait = waits[:maxw]
                newlist.append(inst)
            blk.instructions[:] = newlist
# ---------------------------------------------------------------------------

_NC_CACHE = {}


def build_program():
    if "nc" in _NC_CACHE:
        return _NC_CACHE["nc"]
    f32 = mybir.dt.float32
    bf16 = mybir.dt.bfloat16
    f8 = mybir.dt.float8e4

    nc = bass.Bass()
    # DRAM layouts match the SBUF layouts (partition-major) so every DMA is
    # one long contiguous run per partition.
    # DoubleRow layouts: [p][kk][slot][..] with d = kk*256 + slot*128 + p
    lhsX_d = nc.declare_dram_parameter("lhsX", [128, 2, 2, 512], f8, isOutput=False)
    # first Gram chunk's columns kk-major so the kk=0 slice lands first
    rhs0_d = nc.declare_dram_parameter("rhs0", [2, 128, 2, 1024], f8, isOutput=False)
    rhsX_d = nc.declare_dram_parameter("rhsX", [128, 2, 2, NCOLS - 1024], f8,
                                       isOutput=False)
    sqhl_d = nc.declare_dram_parameter("sqhl", [2, NCOLS], bf16, isOutput=False)
    sqb_d = nc.declare_dram_parameter("sqb", [128, 4], f32, isOutput=False)
    uu_d = nc.declare_dram_parameter("uu", [128, 4, 128], bf16, isOutput=False)
    ee_d = nc.declare_dram_parameter("ee", [44, NCOLS], bf16, isOutput=False)
    out_d = nc.declare_dram_parameter("out", [44, NT], f32, isOutput=True)

    with tile.TileContext(nc) as tc, ExitStack() as ctx:
        singles = ctx.enter_context(tc.tile_pool(name="singles", bufs=1))
        pdist = ctx.enter_context(tc.tile_pool(name="pdist", bufs=4))
        pdmin = ctx.enter_context(tc.tile_pool(name="pdmin", bufs=4))
        pscr = ctx.enter_context(tc.tile_pool(name="pscr", bufs=2))
        pd2 = ctx.enter_context(tc.tile_pool(name="pd2", bufs=3, space="PSUM"))
        pT = ctx.enter_context(tc.tile_pool(name="pT", bufs=2, space="PSUM"))

        # --- DMAs: two serial queues, strictly in need-order. ---
        # scalar queue: lhs weights first, then the small early tensors.
        AX = singles.tile([128, 2, 2, 512], f8)
        nc.scalar.dma_start(out=AX, in_=lhsX_d[:, :, :, :])
        SQ = singles.tile([2, NCOLS], bf16)
        nc.scalar.dma_start(out=SQ, in_=sqhl_d[:, :])
        SB = singles.tile([128, 4], f32)
        nc.scalar.dma_start(out=SB, in_=sqb_d[:, :])
        UU = singles.tile([128, 4, 128], bf16)
        nc.scalar.dma_start(out=UU, in_=uu_d[:, :, :])
        # sync queue: first chunk kk-by-kk, then the rest in need-sized
        # pieces; EE rides between the A-strip and B-strip columns so it
        # never competes with the first Gram chunks.
        BX = singles.tile([128, 2, 2, NCOLS], f8)
        EE = singles.tile([44, NCOLS], bf16)
        for kk in range(2):
            nc.sync.dma_start(
                out=BX[:, kk, :, 0:1024],
                in_=rhs0_d[kk].rearrange("p s j -> p () s j"))
        for (a, b) in ((1024, 1792), (1792, 2304)):
            nc.sync.dma_start(
                out=BX[:, :, :, a:b], in_=rhsX_d[:, :, :, a - 1024:b - 1024])
        nc.sync.dma_start(out=EE, in_=ee_d[:, :])
        for (a, b) in ((2304, 3328), (3328, NCOLS)):
            nc.sync.dma_start(
                out=BX[:, :, :, a:b], in_=rhsX_d[:, :, :, a - 1024:b - 1024])

        ONES = singles.tile([2, 128], bf16)
        nc.vector.memset(ONES, 1.0)
        ACC = singles.tile([44, NT], f32)

        # --- HAM warmup: keep the PE busy while DMAs land so the clock
        # gate opens before the real matmuls start. ---
        WS = singles.tile([128, 640], bf16)
        nc.vector.memset(WS, 0.0)
        wps = pd2.tile([128, 1024], f32, tag="d2")
        for i in range(N_WARM):
            nc.tensor.matmul(
                wps[:, 0:512], WS[:, 0:128], WS[:, 128:640],
                start=(i == 0), stop=(i == N_WARM - 1))

        # --- main: chunk-interleaved pipeline; mask matmuls lag one chunk ---
        def emit_masked(job):
            (si_, coff, spans, sidx, dist, dmin) = job
            for s in sidx:
                so, w, isdiag = spans[s]
                t = si_ * 5 + s
                T = pT.tile([128, 512], f32, tag="T")
                uc = 64 * isdiag
                for mi in range(2):
                    mig = si_ * 2 + mi
                    nc.tensor.matmul(
                        T[0:32, 0:w],
                        UU[:, mig, uc:uc + 32],
                        dist[mi][:, so:so + w],
                        start=(mi == 0), stop=(mi == 1),
                        tile_position=(0, 0), skip_group_check=True)
                    nc.tensor.matmul(
                        T[32:64, 0:w],
                        UU[:, mig, uc + 32:uc + 64],
                        dmin[mi][:, so:so + w],
                        start=(mi == 0), stop=(mi == 1),
                        tile_position=(0, 32), skip_group_check=True)
                scr = pscr.tile([44, 512], f32, tag="scr")
                nc.vector.scalar_tensor_tensor(
                    out=scr[:, 0:w],
                    in0=T[0:44, 0:w],
                    scalar=1.0,
                    in1=EE[:, coff + so:coff + so + w],
                    op0=AluOpType.mult,
                    op1=AluOpType.mult,
                    accum_out=ACC[:, t:t + 1])

        pending = []
        for si_, (coff, scols, spans, chunks) in enumerate(STRIPS):
            dist = {}
            dmin = {}
            for mi in range(2):
                dist[mi] = pdist.tile([128, 2304], bf16, tag="dist",
                                      name=f"dist_{si_}_{mi}")
                dmin[mi] = pdmin.tile([128, 2304], bf16, tag="dmin",
                                      name=f"dmin_{si_}_{mi}")
            for (c0, cw, sidx) in chunks:
                for mi in range(2):
                    mig = si_ * 2 + mi
                    d2 = pd2.tile([128, 1024], f32, tag="d2")
                    for kk in range(2):
                        for s in sidx:
                            so, w, _ = spans[s]
                            nc.tensor.matmul(
                                d2[:, so - c0:so - c0 + w],
                                AX[:, kk, :, mig * 128:(mig + 1) * 128],
                                BX[:, kk, :, coff + so:coff + so + w],
                                start=(kk == 0), stop=False,
                                perf_mode=mybir.MatmulPerfMode.DoubleRow)
                    for s in sidx:
                        so, w, _ = spans[s]
                        nc.tensor.matmul(
                            d2[:, so - c0:so - c0 + w],
                            ONES[0:2, 0:128],
                            SQ[0:2, coff + so:coff + so + w],
                            start=False, stop=True)
                    nc.scalar.activation(
                        out=dist[mi][:, c0:c0 + cw], in_=d2[:, 0:cw],
                        func=mybir.ActivationFunctionType.Sqrt,
                        bias=SB[:, mig:mig + 1], scale=1.0)
                    nc.vector.tensor_scalar_min(
                        dmin[mi][:, c0:c0 + cw], dist[mi][:, c0:c0 + cw], 1.0)
                pending.append((si_, coff, spans, sidx, dist, dmin))
                if len(pending) > 1:
                    emit_masked(pending.pop(0))
        while pending:
            emit_masked(pending.pop(0))
        nc.sync.dma_start(out=out_d[:, :], in_=ACC)

    _split_waits(nc)
    _NC_CACHE["nc"] = nc
    return nc


def prepare_inputs(X, ds, y):
    X = np.asarray(X, dtype=np.float32)
    ds = np.asarray(ds).astype(np.int64)
    y = np.asarray(y).astype(np.int64)

    lhs8 = (-2.0 * X).astype(F8)
    rhs8 = X.astype(F8)
    sq = (-0.5 * (lhs8.astype(np.float64) * rhs8.astype(np.float64)).sum(axis=1))
    sq32 = sq.astype(np.float32)
    sq_hi = sq32.astype(BF16)
    sq_lo = (sq32 - sq_hi.astype(np.float32)).astype(BF16)
    combo = (y * 3 + ds).astype(np.int64)
    cc = np.arange(12) // 3
    aa = np.arange(12) % 3

    def masks(idx):
        # (128, 128): [Usa_off+pad | Us_off+pad | Usa_diag+pad | Us_diag+pad]
        # each mask zero-padded from 12 to 32 cols so the mask matmuls
        # initialize the full 32-partition output groups.
        yo = y[idx, None]
        do = ds[idx, None]
        z = np.zeros((len(idx), 20), dtype=bool)
        usa_o = (yo == cc) & (do != aa)
        us_o = ((yo < cc) & (do < aa)) | ((yo > cc) & (do > aa))
        usa_d = (yo == cc) & (do < aa)
        us_d = (yo < cc) & (do < aa)
        return np.concatenate(
            [usa_o, z, us_o, z, usa_d, z, us_d, z], axis=1).astype(BF16)

    in_maps = []
    for k in range(NCORES):
        rows = []
        cols = []
        for strip, nblk in ((k, 9), (15 - k, 8)):
            rows.append(np.arange(strip * SW, (strip + 1) * SW))
            blk = [(strip + t) % NSTRIP for t in range(1, nblk)] + [strip]
            cols.append(np.concatenate(
                [np.arange(b * SW, (b + 1) * SW) for b in blk]))
        rows = np.concatenate(rows)          # (512,)
        cols = np.concatenate(cols)          # (4352,)

        # DoubleRow layouts: [p][kk][slot][..] with d = kk*256 + slot*128 + p
        lhsX = np.ascontiguousarray(
            lhs8[rows].T.reshape(2, 2, 128, 512).transpose(2, 0, 1, 3))
        rhsT = rhs8[cols].T.reshape(2, 2, 128, NCOLS)
        rhs0 = np.ascontiguousarray(rhsT[:, :, :, 0:1024].transpose(0, 2, 1, 3))
        rhsX = np.ascontiguousarray(rhsT[:, :, :, 1024:].transpose(2, 0, 1, 3))
        sqhl = np.stack([sq_hi[cols], sq_lo[cols]])            # (2, 4352)
        sqb = np.ascontiguousarray(
            (sq32[rows] + np.float32(C0)).reshape(4, 128).T)
        uu = np.ascontiguousarray(np.stack(
            [masks(rows[m * 128:(m + 1) * 128]) for m in range(4)]
        ).transpose(1, 0, 2))
        ee = np.zeros((44, NCOLS), dtype=BF16)
        E = (combo[cols][None, :] == np.arange(12)[:, None])
        ee[0:12] = E
        ee[32:44] = E
        in_maps.append({
            "lhsX": lhsX,
            "rhs0": rhs0,
            "rhsX": rhsX,
            "sqhl": np.ascontiguousarray(sqhl),
            "sqb": sqb.astype(np.float32),
            "uu": uu,
            "ee": ee,
        })
    return in_maps


def finish(results, ds, y, n_classes, n_domains):
    ds = np.asarray(ds).astype(np.int64)
    y = np.asarray(y).astype(np.int64)
    n_classes = int(n_classes)
    n_domains = int(n_domains)
    combo = (y * 3 + ds).astype(np.int64)

    sa_sum = 0.0
    smin_sum = 0.0
    for c in range(NCORES):
        acc = np.asarray(results[c]["out"], dtype=np.float64)   # (44, NT)
        sa_sum += acc[0:12, :].sum()
        smin_sum += acc[32:44, :].sum()

    # exact pair count for the s mask
    cnt = np.bincount(combo, minlength=12).astype(np.float64)
    cc = np.arange(12) // 3
    aa = np.arange(12) % 3
    Ms = ((cc[:, None] < cc[None, :]) & (aa[:, None] < aa[None, :])).astype(np.float64)
    n_pairs_s = cnt @ Ms @ cnt

    n_sa = n_classes * (n_domains * (n_domains - 1) // 2)
    n_s = (n_classes * (n_classes - 1) // 2) * (n_domains * (n_domains - 1) // 2)
    sa_loss = 0.5 * sa_sum / n_sa
    s_loss = 0.5 * (n_pairs_s - smin_sum) / n_s
    return np.array([sa_loss, s_loss], dtype=np.float32)


def run_device(in_maps, trace=False, **kw):
    nc = build_program()
    return run_bass_kernel_spmd(nc, in_maps, core_ids=list(range(NCORES)),
                                trace=trace, **kw)


def kernel(X, ds, y, n_classes, n_domains):
    in_maps = prepare_inputs(X, ds, y)
    res = run_device(in_maps)
    return finish(res.results, ds, y, n_classes, n_domains)


# revision 53
# speedup vs baseline: 1.0049x; 1.0049x over previous
"""JointCCSA loss kernel for 8 Trainium2 NeuronCores — circulant-triangle v3.

reference:
    dists = cdist(X, X)                                  (bs, bs)
    sa_loss = 0.5 * sum[ same_y & ds_lt ] dists / n_sa
    s_loss  = 0.5 * sum[ y_lt  & ds_lt ] relu(1 - dists) / n_s

Strategy (each unordered pair computed ONCE):
  * 16 row strips of 256.  Strip s pairs with col blocks (s+t) mod 16 for
    t=0..8 (t=8 only when s<8): every unordered block pair covered exactly
    once.  Core k owns strips k (9 blocks) and 15-k (8 blocks) -> 17 blocks
    = 4352 cols/core, uniform.  Host rotates columns per core so the device
    program is identical (SPMD).
  * Off-diagonal blocks use symmetrized masks f'(ci,cj)=f(ci,cj)+f(cj,ci)
    (counts both orderings); diagonal blocks the original ordered masks.
    All masks are rank-12 in the (y,ds) combo -> 12-wide matmuls.
  * fp8(e4m3) Gram: d2 = q8(-2x_i)@q8(x_j) + (sqhi_j+sqlo_j via a K=2
    ones-matmul folded into the same PSUM accumulation) + sq_i (ACT bias),
    where sq_i := -0.5*sum q8(-2x_i)*q8(x_i) exactly (f64 on host) so the
    diagonal lands on ~0 (+C0 guard) and sqrt never sees a negative.
  * dist = Sqrt(...) on ScalarE -> bf16; dmin = min(dist,1) on VectorE.
  * Mask reduce: Tsa/Ts accumulate over the two 128-row chunks into one
    PSUM bank via column-tiled concurrent matmuls (sa at col-group 0,
    s at col-group 32).  One scalar_tensor_tensor per span multiplies by
    the one-hot combo mask EE and row-reduces into acc[:, t].
  * Chunk-interleaved emission: mask-matmuls lag the Gram by one chunk so
    the PE never waits on the sqrt/min chain; host sums acc across cores.
"""

import numpy as np
import ml_dtypes
from contextlib import ExitStack

import concourse.bass as bass
import concourse.tile as tile
from concourse import mybir
from concourse.vector_clock import ScopedClock
from concourse.bass_utils import run_bass_kernel_spmd
from concourse.alu_op_type import AluOpType

BS = 4096
D = 512
NCORES = 8
NSTRIP = 16
SW = BS // NSTRIP            # 256 rows per strip
KCH = D // 128               # 4 contraction chunks
NCOLS = 17 * SW              # 4352 cols per core
C0 = 0.0625                  # sqrt-safety bias added into sq_i
F8 = ml_dtypes.float8_e4m3fn
BF16 = ml_dtypes.bfloat16
N_WARM = 6                   # HAM warmup matmuls during the DMA wait

# Per-strip span tables: (col offset rel. to strip, width, is_diag)
SPANS_A = [(0, 512, 0), (512, 512, 0), (1024, 512, 0), (1536, 512, 0),
           (2048, 256, 1)]
SPANS_B = [(0, 512, 0), (512, 512, 0), (1024, 512, 0), (1536, 256, 0),
           (1792, 256, 1)]
# chunks: groups of spans sharing one 2-bank d2 tile (ACT/min granularity)
CHUNKS_A = [(0, 1024, [0, 1]), (1024, 1024, [2, 3]), (2048, 256, [4])]
CHUNKS_B = [(0, 1024, [0, 1]), (1024, 768, [2, 3]), (1792, 256, [4])]
STRIPS = [(0, 2304, SPANS_A, CHUNKS_A), (2304, 2048, SPANS_B, CHUNKS_B)]
NT = 10                      # acc columns (one per span)


# ---------------------------------------------------------------------------
# Patch: this walrus build allows only ONE sync-wait on a CTRL-type (Drain)
# instruction; Tile's final drain aggregates many.  Spread them over
# single-wait SP nops.
def _patched_drain_and_barrier(self, tick_clock, wait_clock):
    nc = self.nc
    coll = nc.sync.nop(nofuse=True, hint="drain_wait_collector")
    wait_clock.add_sem_waits(coll.ins, ScopedClock({None: tick_clock.global_clock}))
    si = coll.ins.sync_info
    waits = list(si.on_wait) if si is not None else []
    if len(waits) > 1:
        si.on_wait = [waits[0]]
        for w in waits[1:]:
            n = nc.sync.nop(nofuse=True, hint="drain_wait_extra")
            n.ins.sync_info = mybir.SyncInfo(on_wait=[w], on_update=[])
    nc.sync.drain()
    nc.all_engine_barrier()
    assert self.sems is not None
    popped = nc._tile_sem_poison_stack.pop()
    assert popped is self._sem_poison
    nc.clear_and_free_semaphores(list(self.sems.allocated().values()))
    nc.all_engine_barrier()


tile.TileContext._drain_and_barrier = _patched_drain_and_barrier


def _split_waits(nc, maxw=1):
    """Hoist extra sync-waits from every instruction onto same-engine NoOps
    (this walrus build rejects instructions with more than ~1 wait)."""
    for fn in nc.m.functions:
        for blk in fn.blocks:
            newlist = []
            for inst in blk.instructions:
                si = getattr(inst, "sync_info", None)
                if si is not None and len(si.on_wait) > maxw:
                    waits = list(si.on_wait)
                    for i, w in enumerate(waits[maxw:]):
                        nop = mybir.InstNoOp(
                            name=f"{inst.name}-wsplit{i}",
                            sync_info=mybir.SyncInfo(on_wait=[w], on_update=[]),
                            bass_nofuse=True,
                            engine=inst.engine,
                        )
                        nc.register_instruction(nop)
                        newlist.append(nop)
                    si.on_wait = waits[:maxw]
                newlist.append(inst)
            blk.instructions[:] = newlist
# ---------------------------------------------------------------------------

_NC_CACHE = {}


def build_program():
    if "nc" in _NC_CACHE:
        return _NC_CACHE["nc"]
    f32 = mybir.dt.float32
    bf16 = mybir.dt.bfloat16
    f8 = mybir.dt.float8e4

    nc = bass.Bass()
    # DRAM layouts match the SBUF layouts (partition-major) so every DMA is
    # one long contiguous run per partition.
    # DoubleRow layouts: [p][kk][slot][..] with d = kk*256 + slot*128 + p
    lhsX_d = nc.declare_dram_parameter("lhsX", [128, 2, 2, 512], f8, isOutput=False)
    # first Gram chunk's columns kk-major so the kk=0 slice lands first
    rhs0_d = nc.declare_dram_parameter("rhs0", [2, 128, 2, 1024], f8, isOutput=False)
    rhsX_d = nc.declare_dram_parameter("rhsX", [128, 2, 2, NCOLS - 1024], f8,
                                       isOutput=False)
    sqhl_d = nc.declare_dram_parameter("sqhl", [2, NCOLS], bf16, isOutput=False)
    sqb_d = nc.declare_dram_parameter("sqb", [128, 4], f32, isOutput=False)
    uu_d = nc.declare_dram_parameter("uu", [128, 4, 128], bf16, isOutput=False)
    ee_d = nc.declare_dram_parameter("ee", [44, NCOLS], bf16, isOutput=False)
    out_d = nc.declare_dram_parameter("out", [44, NT], f32, isOutput=True)

    with tile.TileContext(nc) as tc, ExitStack() as ctx:
        singles = ctx.enter_context(tc.tile_pool(name="singles", bufs=1))
        pdist = ctx.enter_context(tc.tile_pool(name="pdist", bufs=4))
        pdmin = ctx.enter_context(tc.tile_pool(name="pdmin", bufs=4))
        pscr = ctx.enter_context(tc.tile_pool(name="pscr", bufs=2))
        pd2 = ctx.enter_context(tc.tile_pool(name="pd2", bufs=3, space="PSUM"))
        pT = ctx.enter_context(tc.tile_pool(name="pT", bufs=2, space="PSUM"))

        # --- DMAs: two serial queues, strictly in need-order. ---
        # scalar queue: lhs weights first, then the small early tensors.
        AX = singles.tile([128, 2, 2, 512], f8)
        nc.scalar.dma_start(out=AX, in_=lhsX_d[:, :, :, :])
        SQ = singles.tile([2, NCOLS], bf16)
        nc.scalar.dma_start(out=SQ, in_=sqhl_d[:, :])
        SB = singles.tile([128, 4], f32)
        nc.scalar.dma_start(out=SB, in_=sqb_d[:, :])
        UU = singles.tile([128, 4, 128], bf16)
        nc.scalar.dma_start(out=UU, in_=uu_d[:, :, :])
        EE = singles.tile([44, NCOLS], bf16)
        nc.scalar.dma_start(out=EE, in_=ee_d[:, :])
        # sync queue: first chunk kk-by-kk, then the rest in need-sized pieces.
        BX = singles.tile([128, 2, 2, NCOLS], f8)
        for kk in range(2):
            nc.sync.dma_start(
                out=BX[:, kk, :, 0:1024],
                in_=rhs0_d[kk].rearrange("p s j -> p () s j"))
        for (a, b) in ((1024, 1792), (1792, 2304), (2304, 3328), (3328, NCOLS)):
            nc.sync.dma_start(
                out=BX[:, :, :, a:b], in_=rhsX_d[:, :, :, a - 1024:b - 1024])

        ONES = singles.tile([2, 128], bf16)
        nc.vector.memset(ONES, 1.0)
        ACC = singles.tile([44, NT], f32)

        # --- HAM warmup: keep the PE busy while DMAs land so the clock
        # gate opens before the real matmuls start. ---
        WS = singles.tile([128, 640], bf16)
        nc.vector.memset(WS, 0.0)
        wps = pd2.tile([128, 1024], f32, tag="d2")
        for i in range(N_WARM):
            nc.tensor.matmul(
                wps[:, 0:512], WS[:, 0:128], WS[:, 128:640],
                start=(i == 0), stop=(i == N_WARM - 1))

        # --- main: chunk-interleaved pipeline; mask matmuls lag one chunk ---
        def emit_masked(job):
            (si_, coff, spans, sidx, dist, dmin) = job
            for s in sidx:
                so, w, isdiag = spans[s]
                t = si_ * 5 + s
                T = pT.tile([128, 512], f32, tag="T")
                uc = 64 * isdiag
                for mi in range(2):
                    mig = si_ * 2 + mi
                    nc.tensor.matmul(
                        T[0:32, 0:w],
                        UU[:, mig, uc:uc + 32],
                        dist[mi][:, so:so + w],
                        start=(mi == 0), stop=(mi == 1),
                        tile_position=(0, 0), skip_group_check=True)
                    nc.tensor.matmul(
                        T[32:64, 0:w],
                        UU[:, mig, uc + 32:uc + 64],
                        dmin[mi][:, so:so + w],
                        start=(mi == 0), stop=(mi == 1),
                        tile_position=(0, 32), skip_group_check=True)
                scr = pscr.tile([44, 512], f32, tag="scr")
                nc.vector.scalar_tensor_tensor(
                    out=scr[:, 0:w],
                    in0=T[0:44, 0:w],
                    scalar=1.0,
                    in1=EE[:, coff + so:coff + so + w],
                    op0=AluOpType.mult,
                    op1=AluOpType.mult,
                    accum_out=ACC[:, t:t + 1])

        pending = []
        for si_, (coff, scols, spans, chunks) in enumerate(STRIPS):
            dist = {}
            dmin = {}
            for mi in range(2):
                dist[mi] = pdist.tile([128, 2304], bf16, tag="dist",
                                      name=f"dist_{si_}_{mi}")
                dmin[mi] = pdmin.tile([128, 2304], bf16, tag="dmin",
                                      name=f"dmin_{si_}_{mi}")
            for (c0, cw, sidx) in chunks:
                for mi in range(2):
                    mig = si_ * 2 + mi
                    d2 = pd2.tile([128, 1024], f32, tag="d2")
                    for kk in range(2):
                        for s in sidx:
                            so, w, _ = spans[s]
                            nc.tensor.matmul(
                                d2[:, so - c0:so - c0 + w],
                                AX[:, kk, :, mig * 128:(mig + 1) * 128],
                                BX[:, kk, :, coff + so:coff + so + w],
                                start=(kk == 0), stop=False,
                                perf_mode=mybir.MatmulPerfMode.DoubleRow)
                    for s in sidx:
                        so, w, _ = spans[s]
                        nc.tensor.matmul(
                            d2[:, so - c0:so - c0 + w],
                            ONES[0:2, 0:128],
                            SQ[0:2, coff + so:coff + so + w],
                            start=False, stop=True)
                    nc.scalar.activation(
                        out=dist[mi][:, c0:c0 + cw], in_=d2[:, 0:cw],
                        func=mybir.ActivationFunctionType.Sqrt,
                        bias=SB[:, mig:mig + 1], scale=1.0)
                    nc.vector.tensor_scalar_min(
                        dmin[mi][:, c0:c0 + cw], dist[mi][:, c0:c0 + cw], 1.0)
                pending.append((si_, coff, spans, sidx, dist, dmin))
                if len(pending) > 1:
                    emit_masked(pending.pop(0))
        while pending:
            emit_masked(pending.pop(0))
        nc.sync.dma_start(out=out_d[:, :], in_=ACC)

    _split_waits(nc)
    _NC_CACHE["nc"] = nc
    return nc


def prepare_inputs(X, ds, y):
    X = np.asarray(X, dtype=np.float32)
    ds = np.asarray(ds).astype(np.int64)
    y = np.asarray(y).astype(np.int64)

    lhs8 = (-2.0 * X).astype(F8)
    rhs8 = X.astype(F8)
    sq = (-0.5 * (lhs8.astype(np.float64) * rhs8.astype(np.float64)).sum(axis=1))
    sq32 = sq.astype(np.float32)
    sq_hi = sq32.astype(BF16)
    sq_lo = (sq32 - sq_hi.astype(np.float32)).astype(BF16)
    combo = (y * 3 + ds).astype(np.int64)
    cc = np.arange(12) // 3
    aa = np.arange(12) % 3

    def masks(idx):
        # (128, 128): [Usa_off+pad | Us_off+pad | Usa_diag+pad | Us_diag+pad]
        # each mask zero-padded from 12 to 32 cols so the mask matmuls
        # initialize the full 32-partition output groups.
        yo = y[idx, None]
        do = ds[idx, None]
        z = np.zeros((len(idx), 20), dtype=bool)
        usa_o = (yo == cc) & (do != aa)
        us_o = ((yo < cc) & (do < aa)) | ((yo > cc) & (do > aa))
        usa_d = (yo == cc) & (do < aa)
        us_d = (yo < cc) & (do < aa)
        return np.concatenate(
            [usa_o, z, us_o, z, usa_d, z, us_d, z], axis=1).astype(BF16)

    in_maps = []
    for k in range(NCORES):
        rows = []
        cols = []
        for strip, nblk in ((k, 9), (15 - k, 8)):
            rows.append(np.arange(strip * SW, (strip + 1) * SW))
            blk = [(strip + t) % NSTRIP for t in range(1, nblk)] + [strip]
            cols.append(np.concatenate(
                [np.arange(b * SW, (b + 1) * SW) for b in blk]))
        rows = np.concatenate(rows)          # (512,)
        cols = np.concatenate(cols)          # (4352,)

        # DoubleRow layouts: [p][kk][slot][..] with d = kk*256 + slot*128 + p
        lhsX = np.ascontiguousarray(
            lhs8[rows].T.reshape(2, 2, 128, 512).transpose(2, 0, 1, 3))
        rhsT = rhs8[cols].T.reshape(2, 2, 128, NCOLS)
        rhs0 = np.ascontiguousarray(rhsT[:, :, :, 0:1024].transpose(0, 2, 1, 3))
        rhsX = np.ascontiguousarray(rhsT[:, :, :, 1024:].transpose(2, 0, 1, 3))
        sqhl = np.stack([sq_hi[cols], sq_lo[cols]])            # (2, 4352)
        sqb = np.ascontiguousarray(
            (sq32[rows] + np.float32(C0)).reshape(4, 128).T)
        uu = np.ascontiguousarray(np.stack(
            [masks(rows[m * 128:(m + 1) * 128]) for m in range(4)]
        ).transpose(1, 0, 2))
        ee = np.zeros((44, NCOLS), dtype=BF16)
        E = (combo[cols][None, :] == np.arange(12)[:, None])
        ee[0:12] = E
        ee[32:44] = E
        in_maps.append({
            "lhsX": lhsX,
            "rhs0": rhs0,
            "rhsX": rhsX,
            "sqhl": np.ascontiguousarray(sqhl),
            "sqb": sqb.astype(np.float32),
            "uu": uu,
            "ee": ee,
        })
    return in_maps


def finish(results, ds, y, n_classes, n_domains):
    ds = np.asarray(ds).astype(np.int64)
    y = np.asarray(y).astype(np.int64)
    n_classes = int(n_classes)
    n_domains = int(n_domains)
    combo = (y * 3 + ds).astype(np.int64)

    sa_sum = 0.0
    smin_sum = 0.0
    for c in range(NCORES):
        acc = np.asarray(results[c]["out"], dtype=np.float64)   # (44, NT)
        sa_sum += acc[0:12, :].sum()
        smin_sum += acc[32:44, :].sum()

    # exact pair count for the s mask
    cnt = np.bincount(combo, minlength=12).astype(np.float64)
    cc = np.arange(12) // 3
    aa = np.arange(12) % 3
    Ms = ((cc[:, None] < cc[None, :]) & (aa[:, None] < aa[None, :])).astype(np.float64)
    n_pairs_s = cnt @ Ms @ cnt

    n_sa = n_classes * (n_domains * (n_domains - 1) // 2)
    n_s = (n_classes * (n_classes - 1) // 2) * (n_domains * (n_domains - 1) // 2)
    sa_loss = 0.5 * sa_sum / n_sa
    s_loss = 0.5 * (n_pairs_s - smin_sum) / n_s
    return np.array([sa_loss, s_loss], dtype=np.float32)


def run_device(in_maps, trace=False, **kw):
    nc = build_program()
    return run_bass_kernel_spmd(nc, in_maps, core_ids=list(range(NCORES)),
                                trace=trace, **kw)


def kernel(X, ds, y, n_classes, n_domains):
    in_maps = prepare_inputs(X, ds, y)
    res = run_device(in_maps)
    return finish(res.results, ds, y, n_classes, n_domains)
